# revision 28
# baseline (speedup 1.0000x reference)
"""Trainium2 kernel for nn_BasicWHVILinear.

Math (reference):
    qf    = tril(Q) + tril(Q)^T - diag(diag(Q))        (symmetric, 2048x2048)
    Sigma = qf @ qf^T ; L = cholesky(Sigma) ; g = q_mu + L @ eps
    u     = H^T @ (s1 * g)                              (H = scaled Hadamard)
    W     = s2[:,None] * H^T * u[None,:]
    out   = relu(x @ W^T),  x: (16384, 2048)

Key structure: W^T = u[:,None] * H * s2  =>  out^T = relu(s2 ⊙ (H @ (u ⊙ x^T)))
with H = 2048^-1/2 * Hadamard(2048). The dense GEMM is therefore a scaled
Walsh-Hadamard transform. Kronecker-factoring H2048 = H16 (x) H128 turns the
2*2048^3-FLOP GEMM into two thin matmul stages (16x fewer MACs), moving the
kernel from PE-bound (~218 us roofline) to DMA-bound (~46 us).

Sharding: data-parallel over the batch axis, 2048 rows of x per core; the
D-dim parameter pipeline (Sigma -> cholesky -> g -> u, plus the small
Hadamard-factor stationaries) is replicated host-side per the spec hint.

Device pipeline per core (m split into 5 uneven chunks of [64,64,64,48,16]
k-units, a k-unit = 8 m-columns; partition index p = j1*8 + m8, j = j1*128+j2):
  1. u-fence (DVE + GpSimd split): Xs = X_pre ⊙ U_b in place — doubles as the
     DMA fence so downstream PE waits collapse to one semaphore.
  2. mm1: per k-unit matmul(lhsT=Xs_unit[(j1,m8), j2] as the *stationary*,
     rhs=G) with G = kron(Had16, I8)/4. Swapping the operand roles makes the
     output land directly in transposed layout psum[(j2), (i1,m8)] — this
     replaces a separate PE-transpose stage AND its eviction pass. LDWEIGHTS
     overlaps the previous matmul, so a unit costs ~56 ns on the PE.
  3. E_T (Act): evict each filled psum bank -> bf16 SBUF T'[j2, k, (i1,m8)].
  4. mm2 x16: matmul(lhsT=W2_i1[j2,i2] = Had128*2^-3.5*s2-slice, rhs=T'
     windows of fixed i1) -> psum_Y[i2, (k,m8)]. s2 is folded into the 16
     stationaries so the final eviction is a plain relu. Each chunk's mm2/E3
     work is emitted interleaved 1:4 into the NEXT chunk's mm1 stream so the
     PE, Act, and DVE phases overlap across chunk boundaries.
  5. E3 (DVE): tensor_scalar_max(psum, 0) -> bf16 into a 1536-column y ring.
  6. DMA out to outT[(i1*128+i2), m] (1 KiB contiguous runs); the small tail
     chunk keeps the end-of-kernel drain short.

DMA budget is exactly the 8 physical HWDGE queues: [consts + 16 x-units],
[rest of chunk 0], [c1], [c2], [c3+c4] loads (write-once slots, no WAR
hazards) and [y c0+c1], [y c2], [y c3+c4] stores.

Measured: 86.4 us vs the 259.5 us dense-GEMM baseline (3.0x); rel err 5.3e-3
(budget 2e-2). Remaining time is split ~evenly between the DMA floor (~46 us
of HBM traffic), the PSUM-eviction floor on Act+DVE, and fixed framework
preamble/epilogue (~13 us).
"""

import os
import numpy as np

D = 2048
BATCH = 16384
N_CORES = 8
ROWS = BATCH // N_CORES   # 2048 rows of x per core

P = 128
J1 = 16                   # Had16 factor
M8 = 8                    # m-columns packed per partition group
KTOT = ROWS // M8         # 256 k-units per core
KC = 64                   # max k-units per chunk (T' buffer sizing)

CST = 128 + 128 + 16 * 128   # G | U_b | W2[16]  (bf16 cols)
XFREE = KTOT * P             # x free size per partition (32768)

TRACE = bool(int(os.environ.get("WHVI_KERNEL_TRACE", "0")))
LAST_EXEC_TIME_NS = None
LAST_RESULT = None
_PROGRAM = None


def _had(n):
    M = np.array([[1.0]], dtype=np.float64)
    while M.shape[0] < n:
        M = np.block([[M, M], [M, -M]])
    return M


def _host_params(s1, s2, q_mu, q_factor_lower, eps):
    """Replicated parameter pipeline -> (u, s2) then the device stationaries."""
    ql = np.asarray(q_factor_lower, np.float32)
    qf = ql + ql.T - np.diag(np.diag(ql))
    Sigma = qf @ qf.T
    L = np.linalg.cholesky(Sigma)
    g = np.asarray(q_mu, np.float32) + L @ np.asarray(eps, np.float32)
    H = (_had(D) * (D ** -0.5)).astype(np.float32)
    u = H.T @ (np.asarray(s1, np.float32) * g)
    return u.astype(np.float32), np.asarray(s2, np.float32)


def _build_consts(u, s2, bf16):
    """[128, CST] bf16: G | U_b | W2[16] (column-blocks of 128)."""
    H16 = _had(J1)
    H128 = _had(P)
    cst = np.zeros((P, CST), dtype=np.float32)
    # G[(j1,m8), (i1,m8')] = Had16[j1,i1]/4 * delta(m8,m8')
    cst[:, 0:128] = np.kron(H16, np.eye(M8)) * 0.25
    # U_b[(j1,m8), j2] = u[j1*128 + j2]
    cst[:, 128:256] = np.repeat(u.reshape(J1, P), M8, axis=0)
    # W2_i1[j2, i2] = Had128[j2,i2] * 128^-0.5 * s2[i1*128+i2]
    scale = P ** -0.5
    for i1 in range(J1):
        cst[:, 256 + i1 * P : 256 + (i1 + 1) * P] = (
            H128 * scale * s2[i1 * P : (i1 + 1) * P][None, :]
        )
    return cst.astype(bf16)


def _host_xpre(xc, bf16):
    """x core block (2048, 2048) -> [p=(j1,m8), k, j2] bf16, flattened."""
    # row m = k*8+m8, col j = j1*128+j2
    xp = xc.reshape(KTOT, M8, J1, P).transpose(2, 1, 0, 3).reshape(P, KTOT * P)
    return np.ascontiguousarray(xp.astype(bf16))


def _build_program():
    from contextlib import ExitStack

    import concourse.bacc as bacc
    import concourse.mybir as mybir
    import concourse.tile as tile

    f32 = mybir.dt.float32
    bf16 = mybir.dt.bfloat16
    Relu = mybir.ActivationFunctionType.Relu
    Copy = mybir.ActivationFunctionType.Copy

    nc = bacc.Bacc()
    # host packs: [cst (CST cols) | x chunk0 | chunk1 | chunk2 | chunk3]
    xp = nc.declare_dram_parameter("xp", [P, CST + XFREE], bf16, isOutput=False)
    out = nc.declare_dram_parameter("out", [D, ROWS], bf16, isOutput=True)

    # 5 uneven chunks: the small tail chunk shrinks the end-of-kernel drain
    KCS = [64, 64, 64, 48, 16]
    K0S = [0, 64, 128, 192, 240]
    K0A = 16          # first x-DMA covers only 16 units so compute starts early
    YRING = 1536      # yb ring width in m-columns (3 full chunks)

    with tile.TileContext(nc) as tc:
        with ExitStack() as ctx:
            sb = ctx.enter_context(tc.tile_pool(name="sb", bufs=1))
            psum_pool = ctx.enter_context(
                tc.tile_pool(name="psum", bufs=1, space="PSUM")
            )

            # one tensor, write-once x slots (slot c = chunk c) so consts and
            # x chunks can share/combine DMAs with no WAR hazards
            big = sb.tile([P, CST + KTOT * P], bf16)        # 68.6 KB/part
            tp = sb.tile([P, 2, KC, P], bf16)               # T' ping-pong
            yb = sb.tile([P, J1, YRING], bf16)              # y m-ring

            cst = big[:, 0:CST]
            G = cst[:, 0:128]
            Ub = cst[:, 128:256]

            def w2(i1):
                return cst[:, 256 + i1 * P : 256 + (i1 + 1) * P]

            def xsl(c):
                base = CST + K0S[c] * P
                return big[:, base : base + KCS[c] * P].rearrange(
                    "p (k j2) -> p k j2", j2=P
                )

            # psum: 4 banks (16-unit ring) for mm1, 2x2 banks for mm2
            psT = [psum_pool.tile([P, 512], f32, name=f"pt{b}") for b in range(4)]
            psY = [psum_pool.tile([P, 1024], f32, name=f"py{b}") for b in range(2)]

            # --- DMAs: exactly 8 HWDGE queues ---
            # x on SP: [cst+16 units], [rest of c0], [c1], [c2], [c3+c4]
            # y on Act: [c0+c1], [c2], [c3+c4]
            def xdma(e0, e1):
                nc.sync.dma_start(big[:, e0:e1], xp[:, e0:e1])

            xdma(0, CST + K0A * P)
            xdma(CST + K0A * P, CST + 64 * P)
            xdma(CST + 64 * P, CST + 128 * P)
            xdma(CST + 128 * P, CST + 192 * P)
            xdma(CST + 192 * P, CST + KTOT * P)

            out_v = out[:].rearrange("(i1 p) m -> p i1 m", p=P)

            def ufence(xs, k0, k1, pool_share):
                """u-multiply units [k0,k1) in place; DVE head, Pool tail."""
                kd = k1 - pool_share
                ub_d = Ub.unsqueeze(1).broadcast_to([P, kd - k0, P])
                nc.vector.tensor_mul(xs[:, k0:kd, :], xs[:, k0:kd, :], ub_d)
                if pool_share:
                    ub_g = Ub.unsqueeze(1).broadcast_to([P, pool_share, P])
                    nc.gpsimd.tensor_mul(
                        xs[:, kd:k1, :], xs[:, kd:k1, :], ub_g
                    )

            # pending mm2/E3 work carried from the previous chunk, emitted
            # interleaved into the next chunk's mm1 stream so PE/Act/DVE all
            # stay busy across the phase boundary
            pending = []

            def emit_mm2(job, drain=False):
                cc, i1 = job
                ppc = cc % 2
                nn = KCS[cc] * M8
                y0 = (K0S[cc] * M8) % YRING
                half = i1 % 2
                tile_i = (i1 // 2) % 2
                # halves anchor at a fixed 512-col stride: a matmul output
                # must not cross a PSUM bank boundary
                nc.tensor.matmul(
                    psY[tile_i][:, half * 512 : half * 512 + nn],
                    w2(i1),                           # stationary [j2, i2]
                    tp[:, ppc, 0 : KCS[cc], i1 * M8 : (i1 + 1) * M8],
                    start=True,
                    stop=True,
                )
                if half == 1:
                    # E3: relu-evict a psY tile (2 i1 blocks) on DVE; keeping
                    # the whole chain on one engine avoids cross-engine
                    # ping-pong serialization (measured slower when mixed)
                    nc.vector.tensor_scalar_max(
                        yb[:, i1 - 1 : i1 + 1, y0 : y0 + nn],
                        psY[tile_i][:, :].rearrange(
                            "p (a m) -> p a m", a=2
                        )[:, :, 0:nn],
                        0.0,
                    )

            for c in range(len(KCS)):
                pp = c % 2
                xs = xsl(c)
                # u-fence doubles as the DMA fence for this chunk's x data
                if c == 0:
                    ufence(xs, 0, K0A, 0)          # small head: start early
                    ufence(xs, K0A, KCS[c], 12)
                else:
                    ufence(xs, 0, KCS[c], min(12, KCS[c] // 4))

                # mm1 stream with previous chunk's mm2/E3 interleaved 1:4
                for kk in range(KCS[c]):
                    r = kk % 16
                    b = r // 4
                    q = r % 4
                    nc.tensor.matmul(
                        psT[b][:, q * P : (q + 1) * P],
                        xs[:, kk, :],       # stationary [(j1,m8), j2]
                        G,                  # moving     [(j1,m8), (i1,m8)]
                        start=True,
                        stop=True,
                    )
                    if kk % 4 == 3 and pending:
                        emit_mm2(pending.pop(0))
                    # E_T: evict a filled psum bank (4 units) per Act instr
                    if kk % 4 == 3:
                        g4 = kk // 4
                        nc.scalar.activation(
                            tp[:, pp, g4 * 4 : (g4 + 1) * 4, :].rearrange(
                                "p a b -> p (a b)"
                            ),
                            psT[g4 % 4][:, :],
                            Copy,
                        )

                # leftovers from the previous chunk that didn't fit the slots
                while pending:
                    emit_mm2(pending.pop(0), drain=True)
                pending = [(c, i1) for i1 in range(J1)]

                # y stores: emitted only after the producing chunk's E3 work
                # has fully drained (E3[c-1] finishes inside chunk c's stream)
                if c == 2:
                    nc.scalar.dma_start(
                        out_v[:, :, 0:1024], yb[:, :, 0:1024]
                    )
                elif c == 3:
                    nc.scalar.dma_start(
                        out_v[:, :, 1024:1536], yb[:, :, 1024:1536]
                    )

            # final chunk's mm2/E3, then the combined chunk-3+4 store
            while pending:
                emit_mm2(pending.pop(0), drain=True)
            nc.scalar.dma_start(out_v[:, :, 1536:2048], yb[:, :, 0:512])
    nc.finalize()
    return nc


def kernel(x, s1, s2, q_mu, q_factor_lower, eps):
    global _PROGRAM, LAST_EXEC_TIME_NS, LAST_RESULT
    import ml_dtypes
    from concourse.bass_utils import run_bass_kernel_spmd

    bf16 = ml_dtypes.bfloat16
    x = np.asarray(x, np.float32)
    u, s2f = _host_params(s1, s2, q_mu, q_factor_lower, eps)
    cst = _build_consts(u, s2f, bf16)

    if _PROGRAM is None:
        _PROGRAM = _build_program()

    core_ids = list(range(N_CORES))
    in_maps = []
    for c in core_ids:
        xpre = _host_xpre(x[c * ROWS : (c + 1) * ROWS], bf16)
        in_maps.append({"xp": np.ascontiguousarray(np.concatenate([cst, xpre], axis=1))})
    res = run_bass_kernel_spmd(_PROGRAM, in_maps, core_ids, trace=TRACE)
    LAST_RESULT = res
    LAST_EXEC_TIME_NS = res.exec_time_ns
    # device emits outT [i, m] bf16 per core; transpose + upcast on host
    outs = [
        np.asarray(res.results[c]["out"]).astype(np.float32).T for c in core_ids
    ]
    return np.ascontiguousarray(np.concatenate(outs, axis=0))


# revision 29
# speedup vs baseline: 1.0001x; 1.0001x over previous
"""Trainium2 kernel for nn_BasicWHVILinear.

Math (reference):
    qf    = tril(Q) + tril(Q)^T - diag(diag(Q))        (symmetric, 2048x2048)
    Sigma = qf @ qf^T ; L = cholesky(Sigma) ; g = q_mu + L @ eps
    u     = H^T @ (s1 * g)                              (H = scaled Hadamard)
    W     = s2[:,None] * H^T * u[None,:]
    out   = relu(x @ W^T),  x: (16384, 2048)

Key structure: W^T = u[:,None] * H * s2  =>  out^T = relu(s2 ⊙ (H @ (u ⊙ x^T)))
with H = 2048^-1/2 * Hadamard(2048). The dense GEMM is therefore a scaled
Walsh-Hadamard transform. Kronecker-factoring H2048 = H16 (x) H128 turns the
2*2048^3-FLOP GEMM into two thin matmul stages (16x fewer MACs), moving the
kernel from PE-bound (~218 us roofline) to DMA-bound (~46 us).

Sharding: data-parallel over the batch axis, 2048 rows of x per core; the
D-dim parameter pipeline (Sigma -> cholesky -> g -> u, plus the small
Hadamard-factor stationaries) is replicated host-side per the spec hint.

Device pipeline per core (m split into 5 uneven chunks of [64,64,64,48,16]
k-units, a k-unit = 8 m-columns; partition index p = j1*8 + m8, j = j1*128+j2):
  1. u-fence (DVE + GpSimd split): Xs = X_pre ⊙ U_b in place — doubles as the
     DMA fence so downstream PE waits collapse to one semaphore.
  2. mm1: per k-unit matmul(lhsT=Xs_unit[(j1,m8), j2] as the *stationary*,
     rhs=G) with G = kron(Had16, I8)/4. Swapping the operand roles makes the
     output land directly in transposed layout psum[(j2), (i1,m8)] — this
     replaces a separate PE-transpose stage AND its eviction pass. LDWEIGHTS
     overlaps the previous matmul, so a unit costs ~56 ns on the PE.
  3. E_T (Act): evict each filled psum bank -> bf16 SBUF T'[j2, k, (i1,m8)].
  4. mm2 x16: matmul(lhsT=W2_i1[j2,i2] = Had128*2^-3.5*s2-slice, rhs=T'
     windows of fixed i1) -> psum_Y[i2, (k,m8)]. s2 is folded into the 16
     stationaries so the final eviction is a plain relu. Each chunk's mm2/E3
     work is emitted interleaved 1:4 into the NEXT chunk's mm1 stream so the
     PE, Act, and DVE phases overlap across chunk boundaries.
  5. E3 (DVE): tensor_scalar_max(psum, 0) -> bf16 into a 1536-column y ring.
  6. DMA out to outT[(i1*128+i2), m] (1 KiB contiguous runs); the small tail
     chunk keeps the end-of-kernel drain short.

DMA budget is exactly the 8 physical HWDGE queues: [consts + 16 x-units],
[rest of chunk 0], [c1], [c2], [c3+c4] loads (write-once slots, no WAR
hazards) and [y c0+c1], [y c2], [y c3+c4] stores.

Measured: 86.4 us vs the 259.5 us dense-GEMM baseline (3.0x); rel err 5.3e-3
(budget 2e-2). Remaining time is split ~evenly between the DMA floor (~46 us
of HBM traffic), the PSUM-eviction floor on Act+DVE, and fixed framework
preamble/epilogue (~13 us).
"""

import os
import numpy as np

D = 2048
BATCH = 16384
N_CORES = 8
ROWS = BATCH // N_CORES   # 2048 rows of x per core

P = 128
J1 = 16                   # Had16 factor
M8 = 8                    # m-columns packed per partition group
KTOT = ROWS // M8         # 256 k-units per core
KC = 64                   # max k-units per chunk (T' buffer sizing)

CST = 128 + 128 + 16 * 128   # G | U_b | W2[16]  (bf16 cols)
XFREE = KTOT * P             # x free size per partition (32768)

TRACE = bool(int(os.environ.get("WHVI_KERNEL_TRACE", "0")))
LAST_EXEC_TIME_NS = None
LAST_RESULT = None
_PROGRAM = None


def _had(n):
    M = np.array([[1.0]], dtype=np.float64)
    while M.shape[0] < n:
        M = np.block([[M, M], [M, -M]])
    return M


def _host_params(s1, s2, q_mu, q_factor_lower, eps):
    """Replicated parameter pipeline -> (u, s2) then the device stationaries."""
    ql = np.asarray(q_factor_lower, np.float32)
    qf = ql + ql.T - np.diag(np.diag(ql))
    Sigma = qf @ qf.T
    L = np.linalg.cholesky(Sigma)
    g = np.asarray(q_mu, np.float32) + L @ np.asarray(eps, np.float32)
    H = (_had(D) * (D ** -0.5)).astype(np.float32)
    u = H.T @ (np.asarray(s1, np.float32) * g)
    return u.astype(np.float32), np.asarray(s2, np.float32)


def _build_consts(u, s2, bf16):
    """[128, CST] bf16: G | U_b | W2[16] (column-blocks of 128)."""
    H16 = _had(J1)
    H128 = _had(P)
    cst = np.zeros((P, CST), dtype=np.float32)
    # G[(j1,m8), (i1,m8')] = Had16[j1,i1]/4 * delta(m8,m8')
    cst[:, 0:128] = np.kron(H16, np.eye(M8)) * 0.25
    # U_b[(j1,m8), j2] = u[j1*128 + j2]
    cst[:, 128:256] = np.repeat(u.reshape(J1, P), M8, axis=0)
    # W2_i1[j2, i2] = Had128[j2,i2] * 128^-0.5 * s2[i1*128+i2]
    scale = P ** -0.5
    for i1 in range(J1):
        cst[:, 256 + i1 * P : 256 + (i1 + 1) * P] = (
            H128 * scale * s2[i1 * P : (i1 + 1) * P][None, :]
        )
    return cst.astype(bf16)


def _host_xpre(xc, bf16):
    """x core block (2048, 2048) -> [p=(j1,m8), k, j2] bf16, flattened."""
    # row m = k*8+m8, col j = j1*128+j2
    xp = xc.reshape(KTOT, M8, J1, P).transpose(2, 1, 0, 3).reshape(P, KTOT * P)
    return np.ascontiguousarray(xp.astype(bf16))


def _build_program():
    from contextlib import ExitStack

    import concourse.bacc as bacc
    import concourse.mybir as mybir
    import concourse.tile as tile

    f32 = mybir.dt.float32
    bf16 = mybir.dt.bfloat16
    Relu = mybir.ActivationFunctionType.Relu
    Copy = mybir.ActivationFunctionType.Copy

    nc = bacc.Bacc()
    # host packs: [cst (CST cols) | x chunk0 | chunk1 | chunk2 | chunk3]
    xp = nc.declare_dram_parameter("xp", [P, CST + XFREE], bf16, isOutput=False)
    out = nc.declare_dram_parameter("out", [D, ROWS], bf16, isOutput=True)

    # 5 uneven chunks: the small tail chunk shrinks the end-of-kernel drain
    KCS = [64, 64, 64, 48, 16]
    K0S = [0, 64, 128, 192, 240]
    K0A = 16          # first x-DMA covers only 16 units so compute starts early
    YRING = 1536      # yb ring width in m-columns (3 full chunks)

    with tile.TileContext(nc) as tc:
        with ExitStack() as ctx:
            sb = ctx.enter_context(tc.tile_pool(name="sb", bufs=1))
            psum_pool = ctx.enter_context(
                tc.tile_pool(name="psum", bufs=1, space="PSUM")
            )

            # one tensor, write-once x slots (slot c = chunk c) so consts and
            # x chunks can share/combine DMAs with no WAR hazards
            big = sb.tile([P, CST + KTOT * P], bf16)        # 68.6 KB/part
            tp = sb.tile([P, 2, KC, P], bf16)               # T' ping-pong
            yb = sb.tile([P, J1, YRING], bf16)              # y m-ring

            cst = big[:, 0:CST]
            G = cst[:, 0:128]
            Ub = cst[:, 128:256]

            def w2(i1):
                return cst[:, 256 + i1 * P : 256 + (i1 + 1) * P]

            def xsl(c):
                base = CST + K0S[c] * P
                return big[:, base : base + KCS[c] * P].rearrange(
                    "p (k j2) -> p k j2", j2=P
                )

            # psum: 4 banks (16-unit ring) for mm1, 2x2 banks for mm2
            psT = [psum_pool.tile([P, 512], f32, name=f"pt{b}") for b in range(4)]
            psY = [psum_pool.tile([P, 1024], f32, name=f"py{b}") for b in range(2)]

            # --- DMAs: exactly 8 HWDGE queues ---
            # x on SP: [cst+16 units], [rest of c0], [c1], [c2], [c3+c4]
            # y on Act: [c0+c1], [c2], [c3+c4]
            def xdma(e0, e1):
                nc.sync.dma_start(big[:, e0:e1], xp[:, e0:e1])

            xdma(0, CST + K0A * P)
            xdma(CST + K0A * P, CST + 64 * P)
            xdma(CST + 64 * P, CST + 128 * P)
            xdma(CST + 128 * P, CST + 192 * P)
            xdma(CST + 192 * P, CST + KTOT * P)

            out_v = out[:].rearrange("(i1 p) m -> p i1 m", p=P)

            def ufence(xs, k0, k1, pool_share):
                """u-multiply units [k0,k1) in place; DVE head, Pool tail."""
                kd = k1 - pool_share
                ub_d = Ub.unsqueeze(1).broadcast_to([P, kd - k0, P])
                nc.vector.tensor_mul(xs[:, k0:kd, :], xs[:, k0:kd, :], ub_d)
                if pool_share:
                    ub_g = Ub.unsqueeze(1).broadcast_to([P, pool_share, P])
                    nc.gpsimd.tensor_mul(
                        xs[:, kd:k1, :], xs[:, kd:k1, :], ub_g
                    )

            # pending mm2/E3 work carried from the previous chunk, emitted
            # interleaved into the next chunk's mm1 stream so PE/Act/DVE all
            # stay busy across the phase boundary
            pending = []

            def emit_mm2(job, drain=False):
                cc, i1 = job
                ppc = cc % 2
                nn = KCS[cc] * M8
                y0 = (K0S[cc] * M8) % YRING
                half = i1 % 2
                tile_i = (i1 // 2) % 2
                # halves anchor at a fixed 512-col stride: a matmul output
                # must not cross a PSUM bank boundary
                nc.tensor.matmul(
                    psY[tile_i][:, half * 512 : half * 512 + nn],
                    w2(i1),                           # stationary [j2, i2]
                    tp[:, ppc, 0 : KCS[cc], i1 * M8 : (i1 + 1) * M8],
                    start=True,
                    stop=True,
                )
                if half == 1:
                    # E3: relu-evict a psY tile (2 i1 blocks) on DVE; keeping
                    # the whole chain on one engine avoids cross-engine
                    # ping-pong serialization (measured slower when mixed)
                    nc.vector.tensor_scalar_max(
                        yb[:, i1 - 1 : i1 + 1, y0 : y0 + nn],
                        psY[tile_i][:, :].rearrange(
                            "p (a m) -> p a m", a=2
                        )[:, :, 0:nn],
                        0.0,
                    )

            for c in range(len(KCS)):
                pp = c % 2
                xs = xsl(c)
                # u-fence doubles as the DMA fence for this chunk's x data
                if c == 0:
                    ufence(xs, 0, K0A, 0)          # small head: start early
                    ufence(xs, K0A, KCS[c], 12)
                else:
                    ufence(xs, 0, KCS[c], min(20, KCS[c] // 4))

                # mm1 stream with previous chunk's mm2/E3 interleaved 1:4
                for kk in range(KCS[c]):
                    r = kk % 16
                    b = r // 4
                    q = r % 4
                    nc.tensor.matmul(
                        psT[b][:, q * P : (q + 1) * P],
                        xs[:, kk, :],       # stationary [(j1,m8), j2]
                        G,                  # moving     [(j1,m8), (i1,m8)]
                        start=True,
                        stop=True,
                    )
                    if kk % 4 == 3 and pending:
                        emit_mm2(pending.pop(0))
                    # E_T: evict a filled psum bank (4 units) per Act instr
                    if kk % 4 == 3:
                        g4 = kk // 4
                        nc.scalar.activation(
                            tp[:, pp, g4 * 4 : (g4 + 1) * 4, :].rearrange(
                                "p a b -> p (a b)"
                            ),
                            psT[g4 % 4][:, :],
                            Copy,
                        )

                # leftovers from the previous chunk that didn't fit the slots
                while pending:
                    emit_mm2(pending.pop(0), drain=True)
                pending = [(c, i1) for i1 in range(J1)]

                # y stores: emitted only after the producing chunk's E3 work
                # has fully drained (E3[c-1] finishes inside chunk c's stream)
                if c == 2:
                    nc.scalar.dma_start(
                        out_v[:, :, 0:1024], yb[:, :, 0:1024]
                    )
                elif c == 3:
                    nc.scalar.dma_start(
                        out_v[:, :, 1024:1536], yb[:, :, 1024:1536]
                    )

            # final chunk's mm2/E3, then the combined chunk-3+4 store
            while pending:
                emit_mm2(pending.pop(0), drain=True)
            nc.scalar.dma_start(out_v[:, :, 1536:2048], yb[:, :, 0:512])
    nc.finalize()
    return nc


def kernel(x, s1, s2, q_mu, q_factor_lower, eps):
    global _PROGRAM, LAST_EXEC_TIME_NS, LAST_RESULT
    import ml_dtypes
    from concourse.bass_utils import run_bass_kernel_spmd

    bf16 = ml_dtypes.bfloat16
    x = np.asarray(x, np.float32)
    u, s2f = _host_params(s1, s2, q_mu, q_factor_lower, eps)
    cst = _build_consts(u, s2f, bf16)

    if _PROGRAM is None:
        _PROGRAM = _build_program()

    core_ids = list(range(N_CORES))
    in_maps = []
    for c in core_ids:
        xpre = _host_xpre(x[c * ROWS : (c + 1) * ROWS], bf16)
        in_maps.append({"xp": np.ascontiguousarray(np.concatenate([cst, xpre], axis=1))})
    res = run_bass_kernel_spmd(_PROGRAM, in_maps, core_ids, trace=TRACE)
    LAST_RESULT = res
    LAST_EXEC_TIME_NS = res.exec_time_ns
    # device emits outT [i, m] bf16 per core; transpose + upcast on host
    outs = [
        np.asarray(res.results[c]["out"]).astype(np.float32).T for c in core_ids
    ]
    return np.ascontiguousarray(np.concatenate(outs, axis=0))
